# revision 45
# baseline (speedup 1.0000x reference)
"""ALiBi attention (B=4, S=1024, D=1024, H=16) on 8 TRN2 NeuronCores.

Sharding: 8 cores = 4 batches x 2 head-groups (8 heads / 512 hidden each).
Each core computes, for its (batch, head-group):
    QT = wq.T @ xqT          [512, S]   (head-dim-major, "transposed" layout)
    KT = wq.T @ xkT          [512, S]
    V  = xvT.T @ wq          [S, 512]
    per head h:  ST[j,i] = KT_h.T @ QT_h          (scores transposed)
                 P = exp(ST - slope_h * relu(i-j))  (no max-subtract needed)
                 ctxT_h = V_h.T @ P ;  sums = 1^T @ P  (PSUM-accumulated)
                 ctxT_h *= 1/sums  (broadcast along partitions)
    outT = wo.T @ ctxT       [1024, S]  (partial output, transposed, fp16)
Host transposes each core's outT and sums the two head-group partials.

Key structure choices:
 - The two heads of a pair compute scores as K=64 matmuls from
   disjoint partition windows (rows 0:64 / 64:128), which the PE row-
   tiles and runs CONCURRENTLY (~2x on the score stream).
 - Both heads share one wide [128,1024] PSUM score tile so exp runs
   as a single wide ACT op (352-cycle per-op overhead amortized), and
   the exp consumes the score matmuls directly -- no bias stage sits
   inside the 2-deep PSUM score rotation.
 - The ALiBi bias is applied POST-exp as p *= E_h, where E_h[p, c] =
   exp(-s~*relu(c-p)) is a per-head Toeplitz decay table generated
   on-device once (ACT exp over a relu iota table, scale = -s~ from a
   per-core input, since core parity selects the slopes).  The
   16-bit multiply runs at DVE 2x rate; beyond the banding width the
   table is zero, which also kills the dead columns of live tiles.

Schedule: 8 attention groups (pair x i-half); projection chains, V
chunklets and output-projection partials/finals interleave into the
score->exp->PV gaps via a per-jt fill table.  Far sub-diagonal score
tiles where exp underflows are skipped entirely (heads sharded
even/odd across core pairs so the banding savings and the one shared
SPMD program line up).
"""

import math
from contextlib import ExitStack
from functools import partial

import numpy as np

B, S, D = 4, 1024, 1024
H, HD = 16, 64
HL = 8          # heads per core
DL = 512        # local hidden (= HL * HD)
NCORES = 8

_CACHE = {}


def _alibi_slopes(n_head):
    main = 2 ** int(math.log2(n_head))
    m_main = 2.0 ** (-8.0 / main)
    m = m_main ** np.arange(1, 1 + main, dtype=np.float32)
    if main < n_head:
        intra = 2.0 ** (-4.0 / main)
        extra = intra ** np.arange(1, 1 + 2 * (n_head - main), 2, dtype=np.float32)
        m = np.concatenate([m, extra])
    return m.astype(np.float32)


def _build_nc():
    import concourse.bass as bass
    import concourse.mybir as mybir
    import concourse.tile as tile
    from concourse import bacc

    f32 = mybir.dt.float32
    f16 = mybir.dt.float16
    bf16 = mybir.dt.bfloat16
    EXP = mybir.ActivationFunctionType.Exp
    MULT = mybir.AluOpType.mult
    ADD = mybir.AluOpType.add
    MAX = mybir.AluOpType.max

    nc = bacc.Bacc("TRN2", target_bir_lowering=False, debug=False,
                   num_devices=NCORES)

    # All x/w inputs are pre-packed on the host so every DMA line is
    # contiguous per partition (large descriptors, ~3x the landing rate
    # of the strided rearrange loads).
    xq0 = nc.dram_tensor("xq0", [128, 8, 512], f16, kind="ExternalInput").ap()
    xq1 = nc.dram_tensor("xq1", [128, 8, 512], f16, kind="ExternalInput").ap()
    xk0 = nc.dram_tensor("xk0", [128, 8, 512], f16, kind="ExternalInput").ap()
    xk1 = nc.dram_tensor("xk1", [128, 8, 512], f16, kind="ExternalInput").ap()
    xv0 = nc.dram_tensor("xv0", [128, 4, 8, 128], f16, kind="ExternalInput").ap()
    xv1 = nc.dram_tensor("xv1", [128, 4, 8, 128], f16, kind="ExternalInput").ap()
    wq = nc.dram_tensor("wq", [128, 4, 8, 128], f16, kind="ExternalInput").ap()
    wo = nc.dram_tensor("wo", [128, 4, D], f16, kind="ExternalInput").ap()
    negs = nc.dram_tensor("negs", [1, HL], f32, kind="ExternalInput").ap()
    out = nc.dram_tensor("out", [D, S], f16, kind="ExternalOutput").ap()

    # Banded-attention dead-tile table.  ALiBi slope s_h kills any score
    # tile whose minimum (i-j) exceeds T_h = 15/s_h (exp underflow,
    # contribution < ~1e-4 relative).  One SPMD program serves all cores,
    # and heads are sharded even/odd, so a tile is skipped only if dead
    # for BOTH parities (union threshold = the odd head's, always wider).
    # The host orders local heads as globals [4,6,8,10,12,14,0,2](+par)
    # so the most-banded pair lands in the LAST group (short tail).
    slopes_all = _alibi_slopes(H)
    HGLOB = [4, 6, 8, 10, 12, 14, 0, 2]
    t_union = [15.0 / slopes_all[HGLOB[lh] + 1] for lh in range(HL)]

    def tile_dead(lh, jt, ic):
        return (512 * ic - 128 * jt) - 127 > t_union[lh]

    with ExitStack() as ctx:
        tc = ctx.enter_context(tile.TileContext(nc))

        consts = ctx.enter_context(tc.tile_pool(name="consts", bufs=1))
        xvp = ctx.enter_context(tc.tile_pool(name="xvp", bufs=1))
        xsp = ctx.enter_context(tc.tile_pool(name="xsp", bufs=1))
        big = ctx.enter_context(tc.tile_pool(name="big", bufs=1))
        pexp = ctx.enter_context(tc.tile_pool(name="pexp", bufs=3))
        small = ctx.enter_context(tc.tile_pool(name="small", bufs=2))
        accp = ctx.enter_context(tc.tile_pool(name="accp", bufs=1))
        mm_ps = ctx.enter_context(tc.tile_pool(name="mm_ps", bufs=2, space="PSUM"))
        sc_ps = ctx.enter_context(tc.tile_pool(name="sc_ps", bufs=2, space="PSUM"))
        pvs_ps = ctx.enter_context(tc.tile_pool(name="pvs_ps", bufs=1, space="PSUM"))

        # ---- PE warmup: small dummy matmuls (gpsimd memset so they can
        # start as soon as the engine queues open, ~6us) keep the HAM
        # clock-gate lifted until the first real matmul's data lands.
        warm = consts.tile([128, 512], f16, tag="warm")
        nc.gpsimd.memset(warm, 0.0)
        # touch the ACT engine once right away so its activation-table
        # load (~1.3us) happens during the preamble, not at the first exp
        act_dummy = consts.tile([1, 16], f32, tag="act_dummy")
        nc.scalar.activation(act_dummy, warm[0:1, 0:16], EXP)

        def warm_fill(n):
            ps = mm_ps.tile([128, 512], f32, tag="mm")
            for i in range(n):
                nc.tensor.matmul(ps, lhsT=warm[:, 0:128], rhs=warm,
                                 start=(i == 0), stop=(i == n - 1))

        warm_fill(14)

        # ---- input DMAs in need-by order ------------------------------
        wq_sb = consts.tile([128, 4, 8, 128], f16, tag="wq")   # [p][mt][kt][m]

        def load_wq(sl):
            nc.sync.dma_start(out=wq_sb[:, sl, :, :], in_=wq[:, sl, :, :])

        xk_t, xq_t, xv_t = {}, {}, {}

        def load_x(dst, src, half, tag, eng=None):
            t = xsp.tile([128, 8, 512], f16, tag=tag)
            (eng or nc.sync).dma_start(out=t, in_=src)
            dst[half] = t

        def alloc_xv(half):
            xv_t[half] = xvp.tile([128, 4, 8, 128], f16, tag=f"xv{half}",
                                  name=f"xv{half}")

        # Every SBUF-bound DMA costs >=128 descriptors (~2.8us at the
        # ~46 desc/us engine rate), so tensors load whole and the x/v
        # streams split across the sync and gpsimd queues (separate DMA
        # engines process descriptors in parallel).
        alloc_xv(0)
        alloc_xv(1)

        # per-head KT with the two fold rows in the complement; the
        # plain/fold distinction is a K=64 vs K=66 partition window, so
        # no complement zeroing is needed anywhere (rows outside the
        # window are simply never read).
        kt_z = big.tile([128, HL, S], f16, tag="kt")
        qt_z = big.tile([128, HL, S], f16, tag="qt")
        ctx_sb = big.tile([128, 4, S], f16, tag="ctx")

        # wq chunk 0 alone unblocks the pair-0 chains ~2us sooner than a
        # full-wq load; chunks 1:3 follow the critical xk0/xq0 pair
        load_wq(slice(0, 1))
        # xk0 on the scalar queue lands in parallel with wq0 (the ACT
        # queue is otherwise idle until the first exp; its table was
        # preloaded above) -- the kt chain starts ~5us sooner
        load_x(xk_t, xk0, 0, "xk0", eng=nc.scalar)
        # bias fold rows (tiny DMAs, 2 descriptors each) follow xk0 on
        # the otherwise-idle scalar queue; all land well before the
        # first score matmul needs them.  Even local heads keep data in
        # rows 0:64 (bias rows 64:66); odd heads in 64:128 (bias 62:64).
        load_x(xq_t, xq0, 0, "xq0")
        load_wq(slice(1, 4))
        load_x(xk_t, xk1, 1, "xk1")
        load_x(xq_t, xq1, 1, "xq1")
        wo_sb = consts.tile([128, 4, D], f16, tag="wo")        # [c-chunk][ct][o]
        nc.sync.dma_start(out=wo_sb, in_=wo)

        negs_sb = consts.tile([128, HL], f32, tag="negs")
        negs_bcast = bass.AP(tensor=negs.tensor, offset=negs.offset,
                             ap=[[0, 128], [1, HL]])
        nc.gpsimd.dma_start(out=negs_sb, in_=negs_bcast)
        # Toeplitz relu(i-j) bias table, generated on-device: int16 iota
        # (m - p) then max(.,0) into fp16 -- no DMA descriptors burned.
        # MUST precede the xv loads on the gpsimd queue: the first STT
        # needs it ~18us in, while xv issues occupy the queue for ~10us.
        # Extended table [128,1536] with base -512 so a full-width STT is
        # always valid: values are relu'd to 0 above the diagonal, so the
        # bias-add is a no-op there.
        rt_i = consts.tile([128, 1024], mybir.dt.int16, tag="rt_i")
        nc.gpsimd.iota(rt_i, [[1, 1024]], base=0, channel_multiplier=-1)
        rt_sb = consts.tile([128, 1024], f16, tag="rt")
        # the relu goes on DVE: gpsimd tensor ops run ~9ns/elem (14.7us
        # for this tile, measured) and would block the xv DMA issues
        nc.vector.tensor_scalar_max(rt_sb, rt_i, 0)

        # Per-head Toeplitz decay tables E_h[p, c] = exp(-s~ * relu(
        # c-1024-p)): the ALiBi bias is applied POST-exp as p *= E
        # (16-bit DVE multiply at 2x rate), which keeps the score->exp
        # PSUM rotation free of any bias stage -- the exp consumes the
        # score matmul output directly.  Generated on device by one ACT
        # exp over the relu table (scale = -s~ per head, a per-core
        # input, since core parity selects the slopes); beyond the
        # banding width E underflows and the tail is memset to zero,
        # which also zeroes the dead columns of live tiles via the
        # multiply.  Heads 0,1 (first group) and 6,7 (cheap) generate
        # up front during the DMA window; 2..5 ride the fill slots.
        # (columns below the diagonal band start are never read, so the
        # table is rebased: etab[p, c] = exp(-s~ * relu(c - p)))
        etab = big.tile([128, HL, 1024], bf16, tag="etab")
        ew = [min(1024, int(t_union[lh]) + 129) for lh in range(HL)]

        def gen_etab(lh):
            nc.scalar.activation(etab[:, lh, 0:ew[lh]],
                                 rt_sb[:, 0:ew[lh]],
                                 EXP, scale=negs_sb[:, lh:lh + 1])
            if ew[lh] < 1024:
                nc.vector.memset(etab[:, lh, ew[lh]:1024], 0.0)

        for lh in (0, 1, 6, 7):
            gen_etab(lh)
        nc.gpsimd.dma_start(out=xv_t[0][:, 0, :, :], in_=xv0[:, 0, :, :])
        nc.gpsimd.dma_start(out=xv_t[0][:, 1:4, :, :], in_=xv0[:, 1:4, :, :])
        nc.gpsimd.dma_start(out=xv_t[1], in_=xv1)

        # ---- constants / big SBUF tiles -------------------------------
        # V with a ones column per head ([128 s][8 st][8 h][65]); PV and
        # row-sums fuse into one M=65 matmul per head.
        v_sb = big.tile([128, 8, HL, 65], bf16, tag="v")
        ones8 = consts.tile([128, HL], bf16, tag="ones8")
        nc.vector.memset(ones8, 1.0)
        for st in range(8):
            nc.vector.tensor_copy(v_sb[:, st, :, 64], ones8)

        # ---- projection chains ----------------------------------------
        def kt_chain(mt, half):
            ps = mm_ps.tile([128, 512], f32, tag="mm")
            for kt in range(8):
                nc.tensor.matmul(
                    ps,
                    lhsT=wq_sb[:, mt, kt, :],
                    rhs=xk_t[half][:, kt, :],
                    start=(kt == 0), stop=(kt == 7))
            sl = slice(half * 512, (half + 1) * 512)
            nc.vector.tensor_copy(kt_z[0:64, 2 * mt, sl], ps[0:64, :])
            nc.vector.tensor_copy(kt_z[64:128, 2 * mt + 1, sl],
                                  ps[64:128, :])

        def qt_chain(mt, half, first=False):
            ps = mm_ps.tile([128, 512], f32, tag="mm")
            for kt in range(8):
                nc.tensor.matmul(
                    ps,
                    lhsT=wq_sb[:, mt, kt, :],
                    rhs=xq_t[half][:, kt, :],
                    start=(kt == 0), stop=(kt == 7))
            # per head, aligned to the pair rows (head 2mt -> rows 0:64,
            # head 2mt+1 -> rows 64:128; complement rows never read).
            # The very first chain evacuates on DVE: at ~20us the ACT
            # queue is still cold and the first scores wait on this.
            sl = slice(half * 512, (half + 1) * 512)
            nc.vector.tensor_copy(qt_z[0:64, 2 * mt, sl], ps[0:64, :])
            nc.vector.tensor_copy(qt_z[64:128, 2 * mt + 1, sl],
                                  ps[64:128, :])

        def v_chunk(st, g):
            # V projection for (seq-tile st, pair-group g = pairs 2g,2g+1):
            # N=256 keeps LDWEIGHTS (~95ns) hidden behind each matmul
            # (~107ns); N=128 chunks were LDW-bound (+20us PE, measured).
            half, q4 = st // 4, st % 4
            ps = mm_ps.tile([128, 512], f32, tag="mm")
            for kt in range(8):
                nc.tensor.matmul(
                    ps[:, 0:256],
                    lhsT=xv_t[half][:, q4, kt, :],
                    rhs=wq_sb[:, 2 * g:2 * g + 2, kt, :],
                    start=(kt == 0), stop=(kt == 7))
            nc.vector.tensor_copy(
                v_sb[:, st, 4 * g:4 * g + 4, 0:64],
                ps[:, 0:256].rearrange("p (h c) -> p h c", c=64))

        # ---- output projection: partials (pairs 0-2) + finals ---------
        acc_t = {}

        def op_partial(mt, ic):
            ps = mm_ps.tile([128, 512], f32, tag="mm")
            for ct in (0, 1, 2):
                nc.tensor.matmul(
                    ps,
                    lhsT=wo_sb[:, ct, mt * 128:(mt + 1) * 128],
                    rhs=ctx_sb[:, ct, ic * 512:(ic + 1) * 512],
                    start=(ct == 0), stop=(ct == 2))
            acc = accp.tile([128, 512], f16, tag=f"a{ic}{mt}")
            nc.vector.tensor_copy(acc, ps)
            acc_t[(ic, mt)] = acc

        def op_final(mt, ic):
            ps = mm_ps.tile([128, 512], f32, tag="mm")
            nc.tensor.matmul(
                ps,
                lhsT=wo_sb[:, 3, mt * 128:(mt + 1) * 128],
                rhs=ctx_sb[:, 3, ic * 512:(ic + 1) * 512],
                start=True, stop=True)
            st_t = small.tile([128, 512], f16, tag="ostage", bufs=4)
            # fold the SBUF accumulator in during the evacuation (DVE
            # add) instead of an identity matmul on the loaded PE
            nc.vector.tensor_tensor(st_t, in0=ps, in1=acc_t[(ic, mt)],
                                    op=ADD)
            # tail DMAs alternate sync/gpsimd queues so the last
            # (descriptor-bound, ~2.8us) transfers overlap
            q = nc.gpsimd if (ic == 1 and mt % 2 == 1) else nc.sync
            q.dma_start(
                out=out[mt * 128:(mt + 1) * 128, ic * 512:(ic + 1) * 512],
                in_=st_t)

        # ---- attention group ------------------------------------------
        def attn_group(pair, ic, fills=None, fast_norm=False):
            """fills: dict jt -> [callables] interleaved as PE filler.
            fast_norm: skip the pvs SBUF evac; normalize straight out of
            PSUM in per-head pipelined halves (short critical tail)."""
            fills = fills or {}
            hA, hB = 2 * pair, 2 * pair + 1
            i0 = ic * 512
            pvs = pvs_ps.tile([128, 1024], f32, tag="pvs")

            live = [[not tile_dead(h, jt, ic) for h in (hA, hB)]
                    for jt in range(8)]
            sc_tiles = [None] * 8
            # first live jt per head half (banded skipping shifts ic=1
            # starts later; last live jt is always 7)
            first_live = [min(jt for jt in range(8) if live[jt][half])
                          for half in (0, 1)]

            def emit_scores(jt):
                if not (live[jt][0] or live[jt][1]):
                    return
                sc = sc_ps.tile([128, 1024], f32, tag="sc", name="sc")
                jsl = slice(jt * 128, (jt + 1) * 128)
                isl = slice(i0, i0 + 512)
                # The two heads' K=64 score matmuls occupy disjoint row
                # groups (rows 0:64 / 64:128), so the PE runs them
                # CONCURRENTLY (row tiling) -- emit back to back.
                for half, h in ((0, hA), (1, hB)):
                    if not live[jt][half]:
                        continue
                    osl = slice(half * 512, (half + 1) * 512)
                    r = (0, 64) if h % 2 == 0 else (64, 128)
                    nc.tensor.matmul(
                        sc[:, osl],
                        lhsT=kt_z[r[0]:r[1], h, jsl],
                        rhs=qt_z[r[0]:r[1], h, isl],
                        start=True, stop=True)
                sc_tiles[jt] = sc

            emit_scores(0)
            for jt in range(8):
                for f in fills.get(jt, []):
                    f()
                if jt < 7:
                    emit_scores(jt + 1)
                sc = sc_tiles[jt]
                if sc is None:
                    continue
                Dg = i0 - 128 * jt
                p = pexp.tile([128, 1024], bf16, tag="p")
                if live[jt][0] and live[jt][1]:
                    nc.scalar.activation(p, sc, EXP)
                else:
                    off = 0 if live[jt][0] else 512
                    nc.scalar.activation(p[:, off:off + 512],
                                         sc[:, off:off + 512], EXP)
                # post-exp ALiBi decay: p *= E_h on the columns with
                # nonzero bias (16-bit DVE multiply, off the PSUM loop)
                for half, h in ((0, hA), (1, hB)):
                    if not live[jt][half]:
                        continue
                    if Dg > -512:
                        off = half * 512
                        c0 = max(0, -Dg)
                        nc.vector.tensor_tensor(
                            out=p[:, off + c0:off + 512],
                            in0=p[:, off + c0:off + 512],
                            in1=etab[:, h, Dg + c0:512 + Dg],
                            op=MULT)
                for half, h in ((0, hA), (1, hB)):
                    if not live[jt][half]:
                        continue
                    # fused PV + row-sums (M=65: 64 ctx rows + sums row)
                    nc.tensor.matmul(
                        pvs[0:65, half * 512:(half + 1) * 512],
                        lhsT=v_sb[:, jt, h, :],
                        rhs=p[:, half * 512:(half + 1) * 512],
                        start=(jt == first_live[half]), stop=(jt == 7))

            if fast_norm:
                # normalize straight out of PSUM (no evac wait).  Emission
                # order matters: all DVE copies+recips first, then the
                # gpsimd broadcasts, then the TTs -- otherwise a TT waiting
                # on gpsimd stalls the DVE FIFO behind it.
                recip_h, rb_h = [None, None], [None, None]
                for half in (0, 1):
                    cs = slice(half * 512, (half + 1) * 512)
                    sums_h = small.tile([1, 512], f32, tag=f"sums{half}")
                    # ACT is idle right after the last exp; DVE still
                    # drains STT work -- copy sums there
                    nc.scalar.copy(sums_h, pvs[64:65, cs])
                    recip_h[half] = small.tile([1, 512], f32,
                                               tag=f"recip{half}",
                                               name=f"recip{half}")
                    nc.vector.reciprocal_approx_fast(recip_h[half], sums_h)
                for half in (0, 1):
                    rb_h[half] = small.tile([64, 512], f32, tag=f"rb{half}", bufs=1,
                                            name=f"rb{half}")
                    nc.gpsimd.partition_broadcast(rb_h[half], recip_h[half],
                                                  channels=64)
                for half, off in ((0, 0), (1, 64)):
                    cs = slice(half * 512, (half + 1) * 512)
                    nc.vector.tensor_tensor(
                        out=ctx_sb[off:off + 64, pair, i0:i0 + 512],
                        in0=pvs[0:64, cs], in1=rb_h[half], op=MULT)
            else:
                # Evacuate PSUM in one copy so the normalization chain runs
                # off the pvs-reuse critical path.
                pvs_sb = small.tile([65, 1024], f32, tag="pvs_sb", bufs=1)
                nc.vector.tensor_copy(pvs_sb, pvs[0:65, :])
                sums_sb = small.tile([1, 1024], f32, tag="sums")
                nc.vector.tensor_copy(sums_sb, pvs_sb[64:65, :])
                recip = small.tile([1, 1024], f32, tag="recip")
                nc.vector.reciprocal_approx_fast(recip, sums_sb)
                rb = small.tile([64, 1024], f32, tag="rb", bufs=1)
                nc.gpsimd.partition_broadcast(rb, recip, channels=64)
                for half, off in ((0, 0), (1, 64)):
                    nc.vector.tensor_tensor(
                        out=ctx_sb[off:off + 64, pair, i0:i0 + 512],
                        in0=rb[:, half * 512:(half + 1) * 512],
                        in1=pvs_sb[0:64, half * 512:(half + 1) * 512],
                        op=MULT)

        # ---- schedule --------------------------------------------------
        # (host orders heads so local pair 3 is the most banded)
        P = partial
        kt_chain(0, 0)
        qt_chain(0, 0, first=True)

        attn_group(0, 0, {
            0: [P(v_chunk, 0, 0)], 1: [P(v_chunk, 1, 0)],
            2: [P(v_chunk, 2, 0), P(gen_etab, 2)],
            3: [P(v_chunk, 3, 0), P(kt_chain, 0, 1)],
            4: [P(v_chunk, 4, 0)], 5: [P(v_chunk, 5, 0), P(gen_etab, 3)],
            6: [P(v_chunk, 6, 0), P(v_chunk, 7, 0)],
            7: [P(qt_chain, 0, 1)]})
        attn_group(0, 1, {
            0: [P(kt_chain, 1, 0)], 1: [P(v_chunk, 0, 1), P(gen_etab, 4)],
            2: [P(kt_chain, 1, 1)], 3: [P(v_chunk, 1, 1)],
            4: [P(qt_chain, 1, 0)], 5: [P(v_chunk, 2, 1)],
            6: [P(v_chunk, 3, 1)], 7: [P(qt_chain, 1, 1)]})
        attn_group(1, 0, {
            0: [P(v_chunk, 4, 1)], 1: [P(v_chunk, 5, 1), P(gen_etab, 5)],
            2: [P(v_chunk, 6, 1)], 3: [P(v_chunk, 7, 1)],
            4: [P(kt_chain, 2, 0)], 6: [P(kt_chain, 2, 1)]})
        attn_group(1, 1, {
            0: [P(qt_chain, 2, 0)], 2: [P(qt_chain, 2, 1)],
            4: [P(kt_chain, 3, 0)], 6: [P(kt_chain, 3, 1)]})
        attn_group(2, 0, {
            0: [P(qt_chain, 3, 0)], 2: [P(qt_chain, 3, 1)]})
        # output-projection partials start as soon as the needed ctx
        # norms have landed (pairs 0-2 ic0 after group (2,0)'s norm)
        attn_group(2, 1, {
            2: [P(warm_fill, 3)],
            4: [P(warm_fill, 2), P(op_partial, 0, 0)],
            5: [P(op_partial, 1, 0), P(warm_fill, 2)],
            6: [P(warm_fill, 2), P(op_partial, 2, 0)],
            7: [P(op_partial, 3, 0), P(warm_fill, 2)]})
        attn_group(3, 0, {
            0: [P(op_partial, 4, 0)], 1: [P(warm_fill, 2), P(op_partial, 5, 0)],
            2: [P(op_partial, 6, 0)], 3: [P(op_partial, 7, 0)],
            4: [P(op_partial, 0, 1)], 5: [P(op_partial, 1, 1)],
            6: [P(op_partial, 2, 1)], 7: [P(op_partial, 3, 1)]},
            fast_norm=True)
        attn_group(3, 1, {
            0: [P(warm_fill, 4), P(op_partial, 4, 1)],
            1: [P(op_partial, 5, 1)],
            2: [P(op_partial, 6, 1), P(op_final, 0, 0)],
            3: [P(op_partial, 7, 1), P(op_final, 1, 0)],
            4: [P(op_final, 2, 0)], 5: [P(op_final, 3, 0), P(warm_fill, 3)],
            6: [P(op_final, 4, 0), P(op_final, 5, 0)],
            7: [P(op_final, 6, 0), P(op_final, 7, 0)]},
            fast_norm=True)
        # keep the PE streaming through the tail normalization window so
        # the HAM clock-gate stays lifted for the final chains
        warm_fill(8)
        for mt in range(8):
            op_final(mt, 1)

    nc.compile()
    return nc


def _get_nc():
    if "nc" not in _CACHE:
        _CACHE["nc"] = _build_nc()
    return _CACHE["nc"]


def _pack_x(xT, half):
    # [D, S] -> [128, 8, 512] with [p, kt, m] = xT[kt*128+p, half*512+m]
    a = xT.reshape(8, 128, S)[:, :, half * 512:(half + 1) * 512]
    return np.ascontiguousarray(a.transpose(1, 0, 2))


def _pack_xv(xT, half):
    # [D, S] -> [128, 4, 8, 128] with [p, q4, kt, m] =
    #   xT[kt*128+p, half*512 + q4*128 + m]
    a = xT.reshape(8, 128, S)[:, :, half * 512:(half + 1) * 512]
    a = a.reshape(8, 128, 4, 128)
    return np.ascontiguousarray(a.transpose(1, 2, 0, 3))


def _make_in_maps(q, k, v, Wq, Wout):
    q = np.asarray(q, dtype=np.float32)
    k = np.asarray(k, dtype=np.float32)
    v = np.asarray(v, dtype=np.float32)
    Wq = np.asarray(Wq, dtype=np.float32)
    Wout = np.asarray(Wout, dtype=np.float32)

    slopes = _alibi_slopes(H)
    icent = (np.arange(S, dtype=np.float32) - 512.0)

    in_maps = []
    for c in range(NCORES):
        b, par = c // 2, c % 2
        # heads sharded even/odd so the banded-attention savings (small
        # heads have big ALiBi slopes) spread across all cores; ordered
        # so the most-banded pair is processed LAST (shortest tail)
        hsel = [g + par for g in (4, 6, 8, 10, 12, 14, 0, 2)]
        dsel = np.concatenate([np.arange(h * HD, (h + 1) * HD) for h in hsel])
        wq_l = Wq[dsel, :].T.astype(np.float16)        # [D, DL]
        wo_l = Wout[:, dsel].T.astype(np.float16)      # [DL, D]
        qT = q[b].T.astype(np.float16)
        kT = k[b].T.astype(np.float16)
        vT = v[b].T.astype(np.float16)
        # wq packed [p, mt, kt, m] = wq_l[kt*128+p, mt*128+m]
        wq_p = wq_l.reshape(8, 128, 4, 128).transpose(1, 2, 0, 3)
        # wo packed [p, ct, o] = wo_l[ct*128+p, o]
        wo_p = wo_l.reshape(4, 128, D).transpose(1, 0, 2)
        # s~ = fp16-rounded slopes; -s~ feeds the on-device E-table
        # generation as the activation scale (per-core: parity selects
        # the slopes)
        st = slopes[hsel].astype(np.float16).astype(np.float32)  # [HL]
        in_maps.append({
            "xq0": _pack_x(qT, 0), "xq1": _pack_x(qT, 1),
            "xk0": _pack_x(kT, 0), "xk1": _pack_x(kT, 1),
            "xv0": _pack_xv(vT, 0), "xv1": _pack_xv(vT, 1),
            "wq": np.ascontiguousarray(wq_p),
            "wo": np.ascontiguousarray(wo_p),
            "negs": np.ascontiguousarray(-st[None, :]),
        })
    return in_maps


def kernel(q, k, v, mask, Wq, Wout):
    from concourse.bass_utils import run_bass_kernel_spmd

    nc = _get_nc()
    in_maps = _make_in_maps(q, k, v, Wq, Wout)
    res = run_bass_kernel_spmd(nc, in_maps, core_ids=list(range(NCORES)))

    out = np.empty((B, S, D), dtype=np.float32)
    for b in range(B):
        out[b] = (res.results[2 * b]["out"].T.astype(np.float32)
                  + res.results[2 * b + 1]["out"].T.astype(np.float32))
    return out


# revision 46
# speedup vs baseline: 1.0382x; 1.0382x over previous
"""ALiBi attention (B=4, S=1024, D=1024, H=16) on 8 TRN2 NeuronCores.

Sharding: 8 cores = 4 batches x 2 head-groups (8 heads / 512 hidden each).
Each core computes, for its (batch, head-group):
    QT = wq.T @ xqT          [512, S]   (head-dim-major, "transposed" layout)
    KT = wq.T @ xkT          [512, S]
    V  = xvT.T @ wq          [S, 512]
    per head h:  ST[j,i] = KT_h.T @ QT_h          (scores transposed)
                 P = exp(ST - slope_h * relu(i-j))  (no max-subtract needed)
                 ctxT_h = V_h.T @ P ;  sums = 1^T @ P  (PSUM-accumulated)
                 ctxT_h *= 1/sums  (broadcast along partitions)
    outT = wo.T @ ctxT       [1024, S]  (partial output, transposed, fp16)
Host transposes each core's outT and sums the two head-group partials.

Key structure choices:
 - The two heads of a pair compute scores as K=64 matmuls from
   disjoint partition windows (rows 0:64 / 64:128), which the PE row-
   tiles and runs CONCURRENTLY (~2x on the score stream).
 - Both heads share one wide [128,1024] PSUM score tile so exp runs
   as a single wide ACT op (352-cycle per-op overhead amortized), and
   the exp consumes the score matmuls directly -- no bias stage sits
   inside the 2-deep PSUM score rotation.
 - The ALiBi bias is applied POST-exp as p *= E_h, where E_h[p, c] =
   exp(-s~*relu(c-p)) is a per-head Toeplitz decay table generated
   on-device once (ACT exp over a relu iota table, scale = -s~ from a
   per-core input, since core parity selects the slopes).  The
   16-bit multiply runs at DVE 2x rate; beyond the banding width the
   table is zero, which also kills the dead columns of live tiles.

Schedule: 8 attention groups (pair x i-half); projection chains, V
chunklets and output-projection partials/finals interleave into the
score->exp->PV gaps via a per-jt fill table.  Far sub-diagonal score
tiles where exp underflows are skipped entirely (heads sharded
even/odd across core pairs so the banding savings and the one shared
SPMD program line up).
"""

import math
from contextlib import ExitStack
from functools import partial

import numpy as np

B, S, D = 4, 1024, 1024
H, HD = 16, 64
HL = 8          # heads per core
DL = 512        # local hidden (= HL * HD)
NCORES = 8

_CACHE = {}


def _alibi_slopes(n_head):
    main = 2 ** int(math.log2(n_head))
    m_main = 2.0 ** (-8.0 / main)
    m = m_main ** np.arange(1, 1 + main, dtype=np.float32)
    if main < n_head:
        intra = 2.0 ** (-4.0 / main)
        extra = intra ** np.arange(1, 1 + 2 * (n_head - main), 2, dtype=np.float32)
        m = np.concatenate([m, extra])
    return m.astype(np.float32)


def _build_nc():
    import concourse.bass as bass
    import concourse.mybir as mybir
    import concourse.tile as tile
    from concourse import bacc

    f32 = mybir.dt.float32
    f16 = mybir.dt.float16
    bf16 = mybir.dt.bfloat16
    EXP = mybir.ActivationFunctionType.Exp
    MULT = mybir.AluOpType.mult
    ADD = mybir.AluOpType.add
    MAX = mybir.AluOpType.max

    nc = bacc.Bacc("TRN2", target_bir_lowering=False, debug=False,
                   num_devices=NCORES)

    # All x/w inputs are pre-packed on the host so every DMA line is
    # contiguous per partition (large descriptors, ~3x the landing rate
    # of the strided rearrange loads).
    xq0 = nc.dram_tensor("xq0", [128, 8, 512], f16, kind="ExternalInput").ap()
    xq1 = nc.dram_tensor("xq1", [128, 8, 512], f16, kind="ExternalInput").ap()
    xk0 = nc.dram_tensor("xk0", [128, 8, 512], f16, kind="ExternalInput").ap()
    xk1 = nc.dram_tensor("xk1", [128, 8, 512], f16, kind="ExternalInput").ap()
    xv0 = nc.dram_tensor("xv0", [128, 4, 8, 128], f16, kind="ExternalInput").ap()
    xv1 = nc.dram_tensor("xv1", [128, 4, 8, 128], f16, kind="ExternalInput").ap()
    wq = nc.dram_tensor("wq", [128, 4, 8, 128], f16, kind="ExternalInput").ap()
    wo = nc.dram_tensor("wo", [128, 4, D], f16, kind="ExternalInput").ap()
    negs = nc.dram_tensor("negs", [1, HL], f32, kind="ExternalInput").ap()
    out = nc.dram_tensor("out", [D, S], f16, kind="ExternalOutput").ap()

    # Banded-attention dead-tile table.  ALiBi slope s_h kills any score
    # tile whose minimum (i-j) exceeds T_h = 15/s_h (exp underflow,
    # contribution < ~1e-4 relative).  One SPMD program serves all cores,
    # and heads are sharded even/odd, so a tile is skipped only if dead
    # for BOTH parities (union threshold = the odd head's, always wider).
    # The host orders local heads as globals [4,6,8,10,12,14,0,2](+par)
    # so the most-banded pair lands in the LAST group (short tail).
    slopes_all = _alibi_slopes(H)
    HGLOB = [4, 6, 8, 10, 12, 14, 0, 2]
    t_union = [15.0 / slopes_all[HGLOB[lh] + 1] for lh in range(HL)]

    def tile_dead(lh, jt, ic):
        return (512 * ic - 128 * jt) - 127 > t_union[lh]

    with ExitStack() as ctx:
        tc = ctx.enter_context(tile.TileContext(nc))

        consts = ctx.enter_context(tc.tile_pool(name="consts", bufs=1))
        xvp = ctx.enter_context(tc.tile_pool(name="xvp", bufs=1))
        xsp = ctx.enter_context(tc.tile_pool(name="xsp", bufs=1))
        big = ctx.enter_context(tc.tile_pool(name="big", bufs=1))
        pexp = ctx.enter_context(tc.tile_pool(name="pexp", bufs=4))
        small = ctx.enter_context(tc.tile_pool(name="small", bufs=2))
        accp = ctx.enter_context(tc.tile_pool(name="accp", bufs=1))
        mm_ps = ctx.enter_context(tc.tile_pool(name="mm_ps", bufs=2, space="PSUM"))
        sc_ps = ctx.enter_context(tc.tile_pool(name="sc_ps", bufs=2, space="PSUM"))
        pvs_ps = ctx.enter_context(tc.tile_pool(name="pvs_ps", bufs=1, space="PSUM"))

        # ---- PE warmup: small dummy matmuls (gpsimd memset so they can
        # start as soon as the engine queues open, ~6us) keep the HAM
        # clock-gate lifted until the first real matmul's data lands.
        warm = consts.tile([128, 512], f16, tag="warm")
        nc.gpsimd.memset(warm, 0.0)
        # touch the ACT engine once right away so its activation-table
        # load (~1.3us) happens during the preamble, not at the first exp
        act_dummy = consts.tile([1, 16], f32, tag="act_dummy")
        nc.scalar.activation(act_dummy, warm[0:1, 0:16], EXP)

        def warm_fill(n):
            ps = mm_ps.tile([128, 512], f32, tag="mm")
            for i in range(n):
                nc.tensor.matmul(ps, lhsT=warm[:, 0:128], rhs=warm,
                                 start=(i == 0), stop=(i == n - 1))

        warm_fill(14)

        # ---- input DMAs in need-by order ------------------------------
        wq_sb = consts.tile([128, 4, 8, 128], f16, tag="wq")   # [p][mt][kt][m]

        def load_wq(sl):
            nc.sync.dma_start(out=wq_sb[:, sl, :, :], in_=wq[:, sl, :, :])

        xk_t, xq_t, xv_t = {}, {}, {}

        def load_x(dst, src, half, tag, eng=None):
            t = xsp.tile([128, 8, 512], f16, tag=tag)
            (eng or nc.sync).dma_start(out=t, in_=src)
            dst[half] = t

        def alloc_xv(half):
            xv_t[half] = xvp.tile([128, 4, 8, 128], f16, tag=f"xv{half}",
                                  name=f"xv{half}")

        # Every SBUF-bound DMA costs >=128 descriptors (~2.8us at the
        # ~46 desc/us engine rate), so tensors load whole and the x/v
        # streams split across the sync and gpsimd queues (separate DMA
        # engines process descriptors in parallel).
        alloc_xv(0)
        alloc_xv(1)

        # per-head KT with the two fold rows in the complement; the
        # plain/fold distinction is a K=64 vs K=66 partition window, so
        # no complement zeroing is needed anywhere (rows outside the
        # window are simply never read).
        kt_z = big.tile([128, HL, S], f16, tag="kt")
        qt_z = big.tile([128, HL, S], f16, tag="qt")
        ctx_sb = big.tile([128, 4, S], f16, tag="ctx")

        # wq chunk 0 alone unblocks the pair-0 chains ~2us sooner than a
        # full-wq load; chunks 1:3 follow the critical xk0/xq0 pair
        load_wq(slice(0, 1))
        # xk0 on the scalar queue lands in parallel with wq0 (the ACT
        # queue is otherwise idle until the first exp; its table was
        # preloaded above) -- the kt chain starts ~5us sooner
        load_x(xk_t, xk0, 0, "xk0", eng=nc.scalar)
        # bias fold rows (tiny DMAs, 2 descriptors each) follow xk0 on
        # the otherwise-idle scalar queue; all land well before the
        # first score matmul needs them.  Even local heads keep data in
        # rows 0:64 (bias rows 64:66); odd heads in 64:128 (bias 62:64).
        load_x(xq_t, xq0, 0, "xq0")
        load_wq(slice(1, 4))
        load_x(xk_t, xk1, 1, "xk1")
        load_x(xq_t, xq1, 1, "xq1")
        wo_sb = consts.tile([128, 4, D], f16, tag="wo")        # [c-chunk][ct][o]
        nc.sync.dma_start(out=wo_sb, in_=wo)

        negs_sb = consts.tile([128, HL], f32, tag="negs")
        negs_bcast = bass.AP(tensor=negs.tensor, offset=negs.offset,
                             ap=[[0, 128], [1, HL]])
        nc.gpsimd.dma_start(out=negs_sb, in_=negs_bcast)
        # Toeplitz relu(i-j) bias table, generated on-device: int16 iota
        # (m - p) then max(.,0) into fp16 -- no DMA descriptors burned.
        # MUST precede the xv loads on the gpsimd queue: the first STT
        # needs it ~18us in, while xv issues occupy the queue for ~10us.
        # Extended table [128,1536] with base -512 so a full-width STT is
        # always valid: values are relu'd to 0 above the diagonal, so the
        # bias-add is a no-op there.
        rt_i = consts.tile([128, 1024], mybir.dt.int16, tag="rt_i")
        nc.gpsimd.iota(rt_i, [[1, 1024]], base=0, channel_multiplier=-1)
        rt_sb = consts.tile([128, 1024], f16, tag="rt")
        # the relu goes on DVE: gpsimd tensor ops run ~9ns/elem (14.7us
        # for this tile, measured) and would block the xv DMA issues
        nc.vector.tensor_scalar_max(rt_sb, rt_i, 0)

        # Per-head Toeplitz decay tables E_h[p, c] = exp(-s~ * relu(
        # c-1024-p)): the ALiBi bias is applied POST-exp as p *= E
        # (16-bit DVE multiply at 2x rate), which keeps the score->exp
        # PSUM rotation free of any bias stage -- the exp consumes the
        # score matmul output directly.  Generated on device by one ACT
        # exp over the relu table (scale = -s~ per head, a per-core
        # input, since core parity selects the slopes); beyond the
        # banding width E underflows and the tail is memset to zero,
        # which also zeroes the dead columns of live tiles via the
        # multiply.  Heads 0,1 (first group) and 6,7 (cheap) generate
        # up front during the DMA window; 2..5 ride the fill slots.
        # (columns below the diagonal band start are never read, so the
        # table is rebased: etab[p, c] = exp(-s~ * relu(c - p)))
        etab = big.tile([128, HL, 1024], bf16, tag="etab")
        ew = [min(1024, int(t_union[lh]) + 129) for lh in range(HL)]

        def gen_etab(lh):
            nc.scalar.activation(etab[:, lh, 0:ew[lh]],
                                 rt_sb[:, 0:ew[lh]],
                                 EXP, scale=negs_sb[:, lh:lh + 1])
            if ew[lh] < 1024:
                nc.vector.memset(etab[:, lh, ew[lh]:1024], 0.0)

        for lh in (0, 1, 6, 7):
            gen_etab(lh)
        nc.gpsimd.dma_start(out=xv_t[0][:, 0, :, :], in_=xv0[:, 0, :, :])
        nc.gpsimd.dma_start(out=xv_t[0][:, 1:4, :, :], in_=xv0[:, 1:4, :, :])
        nc.gpsimd.dma_start(out=xv_t[1], in_=xv1)

        # ---- constants / big SBUF tiles -------------------------------
        # V with a ones column per head ([128 s][8 st][8 h][65]); PV and
        # row-sums fuse into one M=65 matmul per head.
        v_sb = big.tile([128, 8, HL, 65], bf16, tag="v")
        ones8 = consts.tile([128, HL], bf16, tag="ones8")
        nc.vector.memset(ones8, 1.0)
        for st in range(8):
            nc.vector.tensor_copy(v_sb[:, st, :, 64], ones8)

        # ---- projection chains ----------------------------------------
        def kt_chain(mt, half):
            ps = mm_ps.tile([128, 512], f32, tag="mm")
            for kt in range(8):
                nc.tensor.matmul(
                    ps,
                    lhsT=wq_sb[:, mt, kt, :],
                    rhs=xk_t[half][:, kt, :],
                    start=(kt == 0), stop=(kt == 7))
            sl = slice(half * 512, (half + 1) * 512)
            nc.vector.tensor_copy(kt_z[0:64, 2 * mt, sl], ps[0:64, :])
            nc.vector.tensor_copy(kt_z[64:128, 2 * mt + 1, sl],
                                  ps[64:128, :])

        def qt_chain(mt, half, first=False):
            ps = mm_ps.tile([128, 512], f32, tag="mm")
            for kt in range(8):
                nc.tensor.matmul(
                    ps,
                    lhsT=wq_sb[:, mt, kt, :],
                    rhs=xq_t[half][:, kt, :],
                    start=(kt == 0), stop=(kt == 7))
            # per head, aligned to the pair rows (head 2mt -> rows 0:64,
            # head 2mt+1 -> rows 64:128; complement rows never read).
            # The very first chain evacuates on DVE: at ~20us the ACT
            # queue is still cold and the first scores wait on this.
            sl = slice(half * 512, (half + 1) * 512)
            if first:
                nc.vector.tensor_copy(qt_z[0:64, 2 * mt, sl], ps[0:64, :])
                nc.vector.tensor_copy(qt_z[64:128, 2 * mt + 1, sl],
                                      ps[64:128, :])
            else:
                nc.scalar.copy(qt_z[0:64, 2 * mt, sl], ps[0:64, :])
                nc.scalar.copy(qt_z[64:128, 2 * mt + 1, sl], ps[64:128, :])

        def v_chunk(st, g):
            # V projection for (seq-tile st, pair-group g = pairs 2g,2g+1):
            # N=256 keeps LDWEIGHTS (~95ns) hidden behind each matmul
            # (~107ns); N=128 chunks were LDW-bound (+20us PE, measured).
            half, q4 = st // 4, st % 4
            ps = mm_ps.tile([128, 512], f32, tag="mm")
            for kt in range(8):
                nc.tensor.matmul(
                    ps[:, 0:256],
                    lhsT=xv_t[half][:, q4, kt, :],
                    rhs=wq_sb[:, 2 * g:2 * g + 2, kt, :],
                    start=(kt == 0), stop=(kt == 7))
            if (st + g) % 2 == 0:
                nc.vector.tensor_copy(
                    v_sb[:, st, 4 * g:4 * g + 4, 0:64],
                    ps[:, 0:256].rearrange("p (h c) -> p h c", c=64))
            else:
                nc.scalar.copy(
                    v_sb[:, st, 4 * g:4 * g + 4, 0:64],
                    ps[:, 0:256].rearrange("p (h c) -> p h c", c=64))

        # ---- output projection: partials (pairs 0-2) + finals ---------
        acc_t = {}

        def op_partial(mt, ic):
            ps = mm_ps.tile([128, 512], f32, tag="mm")
            for ct in (0, 1, 2):
                nc.tensor.matmul(
                    ps,
                    lhsT=wo_sb[:, ct, mt * 128:(mt + 1) * 128],
                    rhs=ctx_sb[:, ct, ic * 512:(ic + 1) * 512],
                    start=(ct == 0), stop=(ct == 2))
            acc = accp.tile([128, 512], f16, tag=f"a{ic}{mt}")
            if mt % 2 == 0:
                nc.scalar.copy(acc, ps)
            else:
                nc.vector.tensor_copy(acc, ps)
            acc_t[(ic, mt)] = acc

        def op_final(mt, ic):
            ps = mm_ps.tile([128, 512], f32, tag="mm")
            nc.tensor.matmul(
                ps,
                lhsT=wo_sb[:, 3, mt * 128:(mt + 1) * 128],
                rhs=ctx_sb[:, 3, ic * 512:(ic + 1) * 512],
                start=True, stop=True)
            st_t = small.tile([128, 512], f16, tag="ostage", bufs=4)
            # fold the SBUF accumulator in during the evacuation (DVE
            # add) instead of an identity matmul on the loaded PE
            nc.vector.tensor_tensor(st_t, in0=ps, in1=acc_t[(ic, mt)],
                                    op=ADD)
            # tail DMAs alternate sync/gpsimd queues so the last
            # (descriptor-bound, ~2.8us) transfers overlap
            q = nc.gpsimd if (ic == 1 and mt % 2 == 1) else nc.sync
            q.dma_start(
                out=out[mt * 128:(mt + 1) * 128, ic * 512:(ic + 1) * 512],
                in_=st_t)

        # ---- attention group ------------------------------------------
        def attn_group(pair, ic, fills=None, fast_norm=False):
            """fills: dict jt -> [callables] interleaved as PE filler.
            fast_norm: skip the pvs SBUF evac; normalize straight out of
            PSUM in per-head pipelined halves (short critical tail)."""
            fills = fills or {}
            hA, hB = 2 * pair, 2 * pair + 1
            i0 = ic * 512
            pvs = pvs_ps.tile([128, 1024], f32, tag="pvs")

            live = [[not tile_dead(h, jt, ic) for h in (hA, hB)]
                    for jt in range(8)]
            sc_tiles = [None] * 8
            # first live jt per head half (banded skipping shifts ic=1
            # starts later; last live jt is always 7)
            first_live = [min(jt for jt in range(8) if live[jt][half])
                          for half in (0, 1)]

            def emit_scores(jt):
                if not (live[jt][0] or live[jt][1]):
                    return
                sc = sc_ps.tile([128, 1024], f32, tag="sc", name="sc")
                jsl = slice(jt * 128, (jt + 1) * 128)
                isl = slice(i0, i0 + 512)
                # The two heads' K=64 score matmuls occupy disjoint row
                # groups (rows 0:64 / 64:128), so the PE runs them
                # CONCURRENTLY (row tiling) -- emit back to back.
                for half, h in ((0, hA), (1, hB)):
                    if not live[jt][half]:
                        continue
                    osl = slice(half * 512, (half + 1) * 512)
                    r = (0, 64) if h % 2 == 0 else (64, 128)
                    nc.tensor.matmul(
                        sc[:, osl],
                        lhsT=kt_z[r[0]:r[1], h, jsl],
                        rhs=qt_z[r[0]:r[1], h, isl],
                        start=True, stop=True)
                sc_tiles[jt] = sc

            emit_scores(0)
            for jt in range(8):
                for f in fills.get(jt, []):
                    f()
                if jt < 7:
                    emit_scores(jt + 1)
                sc = sc_tiles[jt]
                if sc is None:
                    continue
                Dg = i0 - 128 * jt
                p = pexp.tile([128, 1024], bf16, tag="p")
                if live[jt][0] and live[jt][1]:
                    nc.scalar.activation(p, sc, EXP)
                else:
                    off = 0 if live[jt][0] else 512
                    nc.scalar.activation(p[:, off:off + 512],
                                         sc[:, off:off + 512], EXP)
                # post-exp ALiBi decay: p *= E_h on the columns with
                # nonzero bias (16-bit DVE multiply, off the PSUM loop)
                for half, h in ((0, hA), (1, hB)):
                    if not live[jt][half]:
                        continue
                    if Dg > -512:
                        off = half * 512
                        c0 = max(0, -Dg)
                        nc.vector.tensor_tensor(
                            out=p[:, off + c0:off + 512],
                            in0=p[:, off + c0:off + 512],
                            in1=etab[:, h, Dg + c0:512 + Dg],
                            op=MULT)
                for half, h in ((0, hA), (1, hB)):
                    if not live[jt][half]:
                        continue
                    # fused PV + row-sums (M=65: 64 ctx rows + sums row)
                    nc.tensor.matmul(
                        pvs[0:65, half * 512:(half + 1) * 512],
                        lhsT=v_sb[:, jt, h, :],
                        rhs=p[:, half * 512:(half + 1) * 512],
                        start=(jt == first_live[half]), stop=(jt == 7))

            if fast_norm:
                # normalize straight out of PSUM (no evac wait).  Emission
                # order matters: all DVE copies+recips first, then the
                # gpsimd broadcasts, then the TTs -- otherwise a TT waiting
                # on gpsimd stalls the DVE FIFO behind it.
                recip_h, rb_h = [None, None], [None, None]
                for half in (0, 1):
                    cs = slice(half * 512, (half + 1) * 512)
                    sums_h = small.tile([1, 512], f32, tag=f"sums{half}")
                    # ACT is idle right after the last exp; DVE still
                    # drains STT work -- copy sums there
                    nc.scalar.copy(sums_h, pvs[64:65, cs])
                    recip_h[half] = small.tile([1, 512], f32,
                                               tag=f"recip{half}",
                                               name=f"recip{half}")
                    nc.vector.reciprocal_approx_fast(recip_h[half], sums_h)
                for half in (0, 1):
                    rb_h[half] = small.tile([64, 512], f32, tag=f"rb{half}", bufs=1,
                                            name=f"rb{half}")
                    nc.gpsimd.partition_broadcast(rb_h[half], recip_h[half],
                                                  channels=64)
                for half, off in ((0, 0), (1, 64)):
                    cs = slice(half * 512, (half + 1) * 512)
                    nc.vector.tensor_tensor(
                        out=ctx_sb[off:off + 64, pair, i0:i0 + 512],
                        in0=pvs[0:64, cs], in1=rb_h[half], op=MULT)
            else:
                # Evacuate PSUM in one copy so the normalization chain runs
                # off the pvs-reuse critical path.
                pvs_sb = small.tile([65, 1024], f32, tag="pvs_sb", bufs=1)
                if ic == 1:   # ic1 groups are DVE-heavy; evacuate via ACT
                    nc.scalar.copy(pvs_sb, pvs[0:65, :])
                else:
                    nc.vector.tensor_copy(pvs_sb, pvs[0:65, :])
                sums_sb = small.tile([1, 1024], f32, tag="sums")
                nc.vector.tensor_copy(sums_sb, pvs_sb[64:65, :])
                recip = small.tile([1, 1024], f32, tag="recip")
                nc.vector.reciprocal_approx_fast(recip, sums_sb)
                rb = small.tile([64, 1024], f32, tag="rb", bufs=1)
                nc.gpsimd.partition_broadcast(rb, recip, channels=64)
                for half, off in ((0, 0), (1, 64)):
                    nc.vector.tensor_tensor(
                        out=ctx_sb[off:off + 64, pair, i0:i0 + 512],
                        in0=rb[:, half * 512:(half + 1) * 512],
                        in1=pvs_sb[0:64, half * 512:(half + 1) * 512],
                        op=MULT)

        # ---- schedule --------------------------------------------------
        # (host orders heads so local pair 3 is the most banded)
        P = partial
        kt_chain(0, 0)
        qt_chain(0, 0, first=True)

        attn_group(0, 0, {
            0: [P(v_chunk, 0, 0)], 1: [P(v_chunk, 1, 0)],
            2: [P(v_chunk, 2, 0), P(gen_etab, 2)],
            3: [P(v_chunk, 3, 0), P(kt_chain, 0, 1)],
            4: [P(v_chunk, 4, 0)], 5: [P(v_chunk, 5, 0), P(gen_etab, 3)],
            6: [P(v_chunk, 6, 0), P(v_chunk, 7, 0)],
            7: [P(qt_chain, 0, 1)]})
        attn_group(0, 1, {
            0: [P(kt_chain, 1, 0)], 1: [P(v_chunk, 0, 1), P(gen_etab, 4)],
            2: [P(kt_chain, 1, 1)], 3: [P(v_chunk, 1, 1)],
            4: [P(qt_chain, 1, 0)], 5: [P(v_chunk, 2, 1)],
            6: [P(v_chunk, 3, 1)], 7: [P(qt_chain, 1, 1)]})
        attn_group(1, 0, {
            0: [P(v_chunk, 4, 1)], 1: [P(v_chunk, 5, 1), P(gen_etab, 5)],
            2: [P(v_chunk, 6, 1)], 3: [P(v_chunk, 7, 1)],
            4: [P(kt_chain, 2, 0)], 6: [P(kt_chain, 2, 1)]})
        attn_group(1, 1, {
            0: [P(qt_chain, 2, 0)], 2: [P(qt_chain, 2, 1)],
            4: [P(kt_chain, 3, 0)], 6: [P(kt_chain, 3, 1)]})
        attn_group(2, 0, {
            0: [P(qt_chain, 3, 0)], 2: [P(qt_chain, 3, 1)]})
        # output-projection partials start as soon as the needed ctx
        # norms have landed (pairs 0-2 ic0 after group (2,0)'s norm)
        attn_group(2, 1, {
            2: [P(warm_fill, 3)],
            4: [P(warm_fill, 2), P(op_partial, 0, 0)],
            5: [P(op_partial, 1, 0), P(warm_fill, 2)],
            6: [P(warm_fill, 2), P(op_partial, 2, 0)],
            7: [P(op_partial, 3, 0), P(warm_fill, 2)]})
        attn_group(3, 0, {
            0: [P(op_partial, 4, 0)], 1: [P(warm_fill, 2), P(op_partial, 5, 0)],
            2: [P(op_partial, 6, 0)], 3: [P(op_partial, 7, 0)],
            4: [P(op_partial, 0, 1)], 5: [P(op_partial, 1, 1)],
            6: [P(op_partial, 2, 1)], 7: [P(op_partial, 3, 1)]},
            fast_norm=True)
        attn_group(3, 1, {
            0: [P(warm_fill, 4), P(op_partial, 4, 1)],
            1: [P(op_partial, 5, 1)],
            2: [P(op_partial, 6, 1), P(op_final, 0, 0)],
            3: [P(op_partial, 7, 1), P(op_final, 1, 0)],
            4: [P(op_final, 2, 0)], 5: [P(op_final, 3, 0), P(warm_fill, 3)],
            6: [P(op_final, 4, 0), P(op_final, 5, 0)],
            7: [P(op_final, 6, 0), P(op_final, 7, 0)]},
            fast_norm=True)
        # keep the PE streaming through the tail normalization window so
        # the HAM clock-gate stays lifted for the final chains
        warm_fill(8)
        for mt in range(8):
            op_final(mt, 1)

    nc.compile()
    return nc


def _get_nc():
    if "nc" not in _CACHE:
        _CACHE["nc"] = _build_nc()
    return _CACHE["nc"]


def _pack_x(xT, half):
    # [D, S] -> [128, 8, 512] with [p, kt, m] = xT[kt*128+p, half*512+m]
    a = xT.reshape(8, 128, S)[:, :, half * 512:(half + 1) * 512]
    return np.ascontiguousarray(a.transpose(1, 0, 2))


def _pack_xv(xT, half):
    # [D, S] -> [128, 4, 8, 128] with [p, q4, kt, m] =
    #   xT[kt*128+p, half*512 + q4*128 + m]
    a = xT.reshape(8, 128, S)[:, :, half * 512:(half + 1) * 512]
    a = a.reshape(8, 128, 4, 128)
    return np.ascontiguousarray(a.transpose(1, 2, 0, 3))


def _make_in_maps(q, k, v, Wq, Wout):
    q = np.asarray(q, dtype=np.float32)
    k = np.asarray(k, dtype=np.float32)
    v = np.asarray(v, dtype=np.float32)
    Wq = np.asarray(Wq, dtype=np.float32)
    Wout = np.asarray(Wout, dtype=np.float32)

    slopes = _alibi_slopes(H)
    icent = (np.arange(S, dtype=np.float32) - 512.0)

    in_maps = []
    for c in range(NCORES):
        b, par = c // 2, c % 2
        # heads sharded even/odd so the banded-attention savings (small
        # heads have big ALiBi slopes) spread across all cores; ordered
        # so the most-banded pair is processed LAST (shortest tail)
        hsel = [g + par for g in (4, 6, 8, 10, 12, 14, 0, 2)]
        dsel = np.concatenate([np.arange(h * HD, (h + 1) * HD) for h in hsel])
        wq_l = Wq[dsel, :].T.astype(np.float16)        # [D, DL]
        wo_l = Wout[:, dsel].T.astype(np.float16)      # [DL, D]
        qT = q[b].T.astype(np.float16)
        kT = k[b].T.astype(np.float16)
        vT = v[b].T.astype(np.float16)
        # wq packed [p, mt, kt, m] = wq_l[kt*128+p, mt*128+m]
        wq_p = wq_l.reshape(8, 128, 4, 128).transpose(1, 2, 0, 3)
        # wo packed [p, ct, o] = wo_l[ct*128+p, o]
        wo_p = wo_l.reshape(4, 128, D).transpose(1, 0, 2)
        # s~ = fp16-rounded slopes; -s~ feeds the on-device E-table
        # generation as the activation scale (per-core: parity selects
        # the slopes)
        st = slopes[hsel].astype(np.float16).astype(np.float32)  # [HL]
        in_maps.append({
            "xq0": _pack_x(qT, 0), "xq1": _pack_x(qT, 1),
            "xk0": _pack_x(kT, 0), "xk1": _pack_x(kT, 1),
            "xv0": _pack_xv(vT, 0), "xv1": _pack_xv(vT, 1),
            "wq": np.ascontiguousarray(wq_p),
            "wo": np.ascontiguousarray(wo_p),
            "negs": np.ascontiguousarray(-st[None, :]),
        })
    return in_maps


def kernel(q, k, v, mask, Wq, Wout):
    from concourse.bass_utils import run_bass_kernel_spmd

    nc = _get_nc()
    in_maps = _make_in_maps(q, k, v, Wq, Wout)
    res = run_bass_kernel_spmd(nc, in_maps, core_ids=list(range(NCORES)))

    out = np.empty((B, S, D), dtype=np.float32)
    for b in range(B):
        out[b] = (res.results[2 * b]["out"].T.astype(np.float32)
                  + res.results[2 * b + 1]["out"].T.astype(np.float32))
    return out


# revision 51
# speedup vs baseline: 1.0612x; 1.0221x over previous
"""ALiBi attention (B=4, S=1024, D=1024, H=16) on 8 TRN2 NeuronCores.

Sharding: 8 cores = 4 batches x 2 head-groups (8 heads / 512 hidden each).
Each core computes, for its (batch, head-group):
    QT = wq.T @ xqT          [512, S]   (head-dim-major, "transposed" layout)
    KT = wq.T @ xkT          [512, S]
    V  = xvT.T @ wq          [S, 512]
    per head h:  ST[j,i] = KT_h.T @ QT_h          (scores transposed)
                 P = exp(ST - slope_h * relu(i-j))  (no max-subtract needed)
                 ctxT_h = V_h.T @ P ;  sums = 1^T @ P  (PSUM-accumulated)
                 ctxT_h *= 1/sums  (broadcast along partitions)
    outT = wo.T @ ctxT       [1024, S]  (partial output, transposed, fp16)
Host transposes each core's outT and sums the two head-group partials.

Key structure choices:
 - The two heads of a pair compute scores as K=64 matmuls from
   disjoint partition windows (rows 0:64 / 64:128), which the PE row-
   tiles and runs CONCURRENTLY (~2x on the score stream).
 - Both heads share one wide [128,1024] PSUM score tile so exp runs
   as a single wide ACT op (352-cycle per-op overhead amortized), and
   the exp consumes the score matmuls directly -- no bias stage sits
   inside the 2-deep PSUM score rotation.
 - The ALiBi bias is applied POST-exp as p *= E_h, where E_h[p, c] =
   exp(-s~*relu(c-p)) is a per-head Toeplitz decay table generated
   on-device once (ACT exp over a relu iota table, scale = -s~ from a
   per-core input, since core parity selects the slopes).  The
   16-bit multiply runs at DVE 2x rate; beyond the banding width the
   table is zero, which also kills the dead columns of live tiles.

Schedule: 8 attention groups (pair x i-half); projection chains, V
chunklets and output-projection partials/finals interleave into the
score->exp->PV gaps via a per-jt fill table.  Far sub-diagonal score
tiles where exp underflows are skipped entirely (heads sharded
even/odd across core pairs so the banding savings and the one shared
SPMD program line up).
"""

import math
from contextlib import ExitStack
from functools import partial

import numpy as np

B, S, D = 4, 1024, 1024
H, HD = 16, 64
HL = 8          # heads per core
DL = 512        # local hidden (= HL * HD)
NCORES = 8

_CACHE = {}


def _alibi_slopes(n_head):
    main = 2 ** int(math.log2(n_head))
    m_main = 2.0 ** (-8.0 / main)
    m = m_main ** np.arange(1, 1 + main, dtype=np.float32)
    if main < n_head:
        intra = 2.0 ** (-4.0 / main)
        extra = intra ** np.arange(1, 1 + 2 * (n_head - main), 2, dtype=np.float32)
        m = np.concatenate([m, extra])
    return m.astype(np.float32)


def _build_nc():
    import concourse.bass as bass
    import concourse.mybir as mybir
    import concourse.tile as tile
    from concourse import bacc

    f32 = mybir.dt.float32
    f16 = mybir.dt.float16
    bf16 = mybir.dt.bfloat16
    EXP = mybir.ActivationFunctionType.Exp
    MULT = mybir.AluOpType.mult
    ADD = mybir.AluOpType.add
    MAX = mybir.AluOpType.max

    nc = bacc.Bacc("TRN2", target_bir_lowering=False, debug=False,
                   num_devices=NCORES)

    # All x/w inputs are pre-packed on the host so every DMA line is
    # contiguous per partition (large descriptors, ~3x the landing rate
    # of the strided rearrange loads).
    xq0 = nc.dram_tensor("xq0", [128, 8, 512], f16, kind="ExternalInput").ap()
    xq1 = nc.dram_tensor("xq1", [128, 8, 512], f16, kind="ExternalInput").ap()
    xk0 = nc.dram_tensor("xk0", [128, 8, 512], f16, kind="ExternalInput").ap()
    xk1 = nc.dram_tensor("xk1", [128, 8, 512], f16, kind="ExternalInput").ap()
    xv0 = nc.dram_tensor("xv0", [128, 4, 8, 128], f16, kind="ExternalInput").ap()
    xv1 = nc.dram_tensor("xv1", [128, 4, 8, 128], f16, kind="ExternalInput").ap()
    wq = nc.dram_tensor("wq", [128, 4, 8, 128], f16, kind="ExternalInput").ap()
    wo = nc.dram_tensor("wo", [128, 4, D], f16, kind="ExternalInput").ap()
    negs = nc.dram_tensor("negs", [1, HL], f32, kind="ExternalInput").ap()
    out = nc.dram_tensor("out", [D, S], f16, kind="ExternalOutput").ap()

    # Banded-attention dead-tile table.  ALiBi slope s_h kills any score
    # tile whose minimum (i-j) exceeds T_h = 15/s_h (exp underflow,
    # contribution < ~1e-4 relative).  One SPMD program serves all cores,
    # and heads are sharded even/odd, so a tile is skipped only if dead
    # for BOTH parities (union threshold = the odd head's, always wider).
    # The host orders local heads as globals [4,6,8,10,12,14,0,2](+par)
    # so the most-banded pair lands in the LAST group (short tail).
    slopes_all = _alibi_slopes(H)
    HGLOB = [4, 6, 8, 10, 12, 14, 0, 2]
    t_union = [15.0 / slopes_all[HGLOB[lh] + 1] for lh in range(HL)]

    def tile_dead(lh, jt, ic):
        return (512 * ic - 128 * jt) - 127 > t_union[lh]

    with ExitStack() as ctx:
        tc = ctx.enter_context(tile.TileContext(nc))

        consts = ctx.enter_context(tc.tile_pool(name="consts", bufs=1))
        xvp = ctx.enter_context(tc.tile_pool(name="xvp", bufs=1))
        xsp = ctx.enter_context(tc.tile_pool(name="xsp", bufs=1))
        big = ctx.enter_context(tc.tile_pool(name="big", bufs=1))
        pexp = ctx.enter_context(tc.tile_pool(name="pexp", bufs=4))
        small = ctx.enter_context(tc.tile_pool(name="small", bufs=2))
        accp = ctx.enter_context(tc.tile_pool(name="accp", bufs=1))
        mm_ps = ctx.enter_context(tc.tile_pool(name="mm_ps", bufs=2, space="PSUM"))
        sc_ps = ctx.enter_context(tc.tile_pool(name="sc_ps", bufs=2, space="PSUM"))
        pvs_ps = ctx.enter_context(tc.tile_pool(name="pvs_ps", bufs=1, space="PSUM"))

        # ---- PE warmup: small dummy matmuls (gpsimd memset so they can
        # start as soon as the engine queues open, ~6us) keep the HAM
        # clock-gate lifted until the first real matmul's data lands.
        warm = consts.tile([128, 512], f16, tag="warm")
        nc.gpsimd.memset(warm, 0.0)
        # touch the ACT engine once right away so its activation-table
        # load (~1.3us) happens during the preamble, not at the first exp
        act_dummy = consts.tile([1, 16], f32, tag="act_dummy")
        nc.scalar.activation(act_dummy, warm[0:1, 0:16], EXP)

        def warm_fill(n):
            ps = mm_ps.tile([128, 512], f32, tag="mm")
            for i in range(n):
                nc.tensor.matmul(ps, lhsT=warm[:, 0:128], rhs=warm,
                                 start=(i == 0), stop=(i == n - 1))

        warm_fill(14)

        # ---- input DMAs in need-by order ------------------------------
        wq_sb = consts.tile([128, 4, 8, 128], f16, tag="wq")   # [p][mt][kt][m]

        def load_wq(sl):
            nc.sync.dma_start(out=wq_sb[:, sl, :, :], in_=wq[:, sl, :, :])

        xk_t, xq_t, xv_t = {}, {}, {}

        def load_x(dst, src, half, tag, eng=None):
            t = xsp.tile([128, 8, 512], f16, tag=tag)
            (eng or nc.sync).dma_start(out=t, in_=src)
            dst[half] = t

        def alloc_xv(half):
            xv_t[half] = xvp.tile([128, 4, 8, 128], f16, tag=f"xv{half}",
                                  name=f"xv{half}")

        # Every SBUF-bound DMA costs >=128 descriptors (~2.8us at the
        # ~46 desc/us engine rate), so tensors load whole and the x/v
        # streams split across the sync and gpsimd queues (separate DMA
        # engines process descriptors in parallel).
        alloc_xv(0)
        alloc_xv(1)

        # per-head KT with the two fold rows in the complement; the
        # plain/fold distinction is a K=64 vs K=66 partition window, so
        # no complement zeroing is needed anywhere (rows outside the
        # window are simply never read).
        kt_z = big.tile([128, HL, S], f16, tag="kt")
        qt_z = big.tile([128, HL, S], f16, tag="qt")
        ctx_sb = big.tile([128, 4, S], f16, tag="ctx")

        # wq chunk 0 alone unblocks the pair-0 chains ~2us sooner than a
        # full-wq load; chunks 1:3 follow the critical xk0/xq0 pair
        load_wq(slice(0, 1))
        # xk0 on the scalar queue lands in parallel with wq0 (the ACT
        # queue is otherwise idle until the first exp; its table was
        # preloaded above) -- the kt chain starts ~5us sooner
        load_x(xk_t, xk0, 0, "xk0", eng=nc.scalar)
        # bias fold rows (tiny DMAs, 2 descriptors each) follow xk0 on
        # the otherwise-idle scalar queue; all land well before the
        # first score matmul needs them.  Even local heads keep data in
        # rows 0:64 (bias rows 64:66); odd heads in 64:128 (bias 62:64).
        load_x(xq_t, xq0, 0, "xq0")
        load_wq(slice(1, 4))
        load_x(xk_t, xk1, 1, "xk1")
        load_x(xq_t, xq1, 1, "xq1")
        wo_sb = consts.tile([128, 4, D], f16, tag="wo")        # [c-chunk][ct][o]
        nc.sync.dma_start(out=wo_sb, in_=wo)

        negs_sb = consts.tile([128, HL], f32, tag="negs")
        negs_bcast = bass.AP(tensor=negs.tensor, offset=negs.offset,
                             ap=[[0, 128], [1, HL]])
        nc.gpsimd.dma_start(out=negs_sb, in_=negs_bcast)
        # Toeplitz relu(i-j) bias table, generated on-device: int16 iota
        # (m - p) then max(.,0) into fp16 -- no DMA descriptors burned.
        # MUST precede the xv loads on the gpsimd queue: the first STT
        # needs it ~18us in, while xv issues occupy the queue for ~10us.
        # Extended table [128,1536] with base -512 so a full-width STT is
        # always valid: values are relu'd to 0 above the diagonal, so the
        # bias-add is a no-op there.
        rt_i = consts.tile([128, 1024], mybir.dt.int16, tag="rt_i")
        nc.gpsimd.iota(rt_i, [[1, 1024]], base=0, channel_multiplier=-1)
        rt_sb = consts.tile([128, 1024], f16, tag="rt")
        # the relu goes on DVE: gpsimd tensor ops run ~9ns/elem (14.7us
        # for this tile, measured) and would block the xv DMA issues
        nc.vector.tensor_scalar_max(rt_sb, rt_i, 0)

        # Per-head Toeplitz decay tables E_h[p, c] = exp(-s~ * relu(
        # c-1024-p)): the ALiBi bias is applied POST-exp as p *= E
        # (16-bit DVE multiply at 2x rate), which keeps the score->exp
        # PSUM rotation free of any bias stage -- the exp consumes the
        # score matmul output directly.  Generated on device by one ACT
        # exp over the relu table (scale = -s~ per head, a per-core
        # input, since core parity selects the slopes); beyond the
        # banding width E underflows and the tail is memset to zero,
        # which also zeroes the dead columns of live tiles via the
        # multiply.  Heads 0,1 (first group) and 6,7 (cheap) generate
        # up front during the DMA window; 2..5 ride the fill slots.
        # (columns below the diagonal band start are never read, so the
        # table is rebased: etab[p, c] = exp(-s~ * relu(c - p)))
        etab = big.tile([128, HL, 1024], bf16, tag="etab")
        ew = [min(1024, int(t_union[lh]) + 129) for lh in range(HL)]

        def gen_etab(lh):
            nc.scalar.activation(etab[:, lh, 0:ew[lh]],
                                 rt_sb[:, 0:ew[lh]],
                                 EXP, scale=negs_sb[:, lh:lh + 1])
            if ew[lh] < 1024:
                nc.vector.memset(etab[:, lh, ew[lh]:1024], 0.0)

        for lh in (0, 1, 6, 7):
            gen_etab(lh)
        nc.gpsimd.dma_start(out=xv_t[0][:, 0, :, :], in_=xv0[:, 0, :, :])
        nc.gpsimd.dma_start(out=xv_t[0][:, 1:4, :, :], in_=xv0[:, 1:4, :, :])
        nc.gpsimd.dma_start(out=xv_t[1], in_=xv1)

        # ---- constants / big SBUF tiles -------------------------------
        # V with a ones column per head ([128 s][8 st][8 h][65]); PV and
        # row-sums fuse into one M=65 matmul per head.
        v_sb = big.tile([128, 8, HL, 65], bf16, tag="v")
        ones8 = consts.tile([128, HL], bf16, tag="ones8")
        nc.vector.memset(ones8, 1.0)
        for st in range(8):
            nc.vector.tensor_copy(v_sb[:, st, :, 64], ones8)

        # ---- projection chains ----------------------------------------
        def kt_chain(mt, half):
            ps = mm_ps.tile([128, 512], f32, tag="mm")
            for kt in range(8):
                nc.tensor.matmul(
                    ps,
                    lhsT=wq_sb[:, mt, kt, :],
                    rhs=xk_t[half][:, kt, :],
                    start=(kt == 0), stop=(kt == 7))
            sl = slice(half * 512, (half + 1) * 512)
            nc.vector.tensor_copy(kt_z[0:64, 2 * mt, sl], ps[0:64, :])
            nc.vector.tensor_copy(kt_z[64:128, 2 * mt + 1, sl],
                                  ps[64:128, :])

        def qt_chain(mt, half, first=False):
            ps = mm_ps.tile([128, 512], f32, tag="mm")
            for kt in range(8):
                nc.tensor.matmul(
                    ps,
                    lhsT=wq_sb[:, mt, kt, :],
                    rhs=xq_t[half][:, kt, :],
                    start=(kt == 0), stop=(kt == 7))
            # per head, aligned to the pair rows (head 2mt -> rows 0:64,
            # head 2mt+1 -> rows 64:128; complement rows never read).
            # The very first chain evacuates on DVE: at ~20us the ACT
            # queue is still cold and the first scores wait on this.
            sl = slice(half * 512, (half + 1) * 512)
            if first:
                nc.vector.tensor_copy(qt_z[0:64, 2 * mt, sl], ps[0:64, :])
                nc.vector.tensor_copy(qt_z[64:128, 2 * mt + 1, sl],
                                      ps[64:128, :])
            else:
                nc.scalar.copy(qt_z[0:64, 2 * mt, sl], ps[0:64, :])
                nc.scalar.copy(qt_z[64:128, 2 * mt + 1, sl], ps[64:128, :])

        def v_chunk(st, g):
            # V projection for (seq-tile st, pair-group g = pairs 2g,2g+1):
            # N=256 keeps LDWEIGHTS (~95ns) hidden behind each matmul
            # (~107ns); N=128 chunks were LDW-bound (+20us PE, measured).
            half, q4 = st // 4, st % 4
            ps = mm_ps.tile([128, 512], f32, tag="mm")
            for kt in range(8):
                nc.tensor.matmul(
                    ps[:, 0:256],
                    lhsT=xv_t[half][:, q4, kt, :],
                    rhs=wq_sb[:, 2 * g:2 * g + 2, kt, :],
                    start=(kt == 0), stop=(kt == 7))
            if (st + g) % 2 == 0:
                nc.vector.tensor_copy(
                    v_sb[:, st, 4 * g:4 * g + 4, 0:64],
                    ps[:, 0:256].rearrange("p (h c) -> p h c", c=64))
            else:
                nc.scalar.copy(
                    v_sb[:, st, 4 * g:4 * g + 4, 0:64],
                    ps[:, 0:256].rearrange("p (h c) -> p h c", c=64))

        # ---- output projection: partials (pairs 0-2) + finals ---------
        acc_t = {}

        def op_partial(mt, ic):
            ps = mm_ps.tile([128, 512], f32, tag="mm")
            for ct in (0, 1, 2):
                nc.tensor.matmul(
                    ps,
                    lhsT=wo_sb[:, ct, mt * 128:(mt + 1) * 128],
                    rhs=ctx_sb[:, ct, ic * 512:(ic + 1) * 512],
                    start=(ct == 0), stop=(ct == 2))
            acc = accp.tile([128, 512], f16, tag=f"a{ic}{mt}")
            if mt % 2 == 0:
                nc.scalar.copy(acc, ps)
            else:
                nc.vector.tensor_copy(acc, ps)
            acc_t[(ic, mt)] = acc

        def op_final(mt, ic):
            ps = mm_ps.tile([128, 512], f32, tag="mm")
            nc.tensor.matmul(
                ps,
                lhsT=wo_sb[:, 3, mt * 128:(mt + 1) * 128],
                rhs=ctx_sb[:, 3, ic * 512:(ic + 1) * 512],
                start=True, stop=True)
            st_t = small.tile([128, 512], f16, tag="ostage", bufs=4)
            # fold the SBUF accumulator in during the evacuation (DVE
            # add) instead of an identity matmul on the loaded PE
            nc.vector.tensor_tensor(st_t, in0=ps, in1=acc_t[(ic, mt)],
                                    op=ADD)
            # tail DMAs alternate sync/gpsimd queues so the last
            # (descriptor-bound, ~2.8us) transfers overlap
            q = nc.gpsimd if (ic == 1 and mt % 2 == 1) else nc.sync
            q.dma_start(
                out=out[mt * 128:(mt + 1) * 128, ic * 512:(ic + 1) * 512],
                in_=st_t)

        # ---- attention group ------------------------------------------
        def attn_group(pair, ic, fills=None, fast_norm=False):
            """fills: dict jt -> [callables] interleaved as PE filler.
            fast_norm: skip the pvs SBUF evac; normalize straight out of
            PSUM in per-head pipelined halves (short critical tail)."""
            fills = fills or {}
            hA, hB = 2 * pair, 2 * pair + 1
            i0 = ic * 512
            pvs = pvs_ps.tile([128, 1024], f32, tag="pvs")

            # live width per (jt, half): columns beyond the ALiBi band
            # (E underflows to 0 for the whole j-tile) are trimmed from
            # the score matmul, exp, decay multiply and PV -- rounded up
            # to 128 so the PV accumulation regions stay nested.
            def width(jt, h):
                wr = 128 * jt + 128 + t_union[h] - i0
                if wr <= 0:
                    return 0
                return min(512, 128 * -(-int(wr) // 128))

            Ws = [[width(jt, h) for h in (hA, hB)] for jt in range(8)]
            live = [[w > 0 for w in Ws[jt]] for jt in range(8)]
            sc_tiles = [None] * 8
            # first live jt per head half (banded skipping shifts ic=1
            # starts later; last live jt is always 7)
            first_live = [min(jt for jt in range(8) if live[jt][half])
                          for half in (0, 1)]

            def emit_scores(jt):
                if not (live[jt][0] or live[jt][1]):
                    return
                sc = sc_ps.tile([128, 1024], f32, tag="sc", name="sc")
                jsl = slice(jt * 128, (jt + 1) * 128)
                # The two heads' K=64 score matmuls occupy disjoint row
                # groups (rows 0:64 / 64:128), so the PE runs them
                # CONCURRENTLY (row tiling) -- emit back to back.
                for half, h in ((0, hA), (1, hB)):
                    w = Ws[jt][half]
                    if w == 0:
                        continue
                    off = half * 512
                    r = (0, 64) if h % 2 == 0 else (64, 128)
                    nc.tensor.matmul(
                        sc[:, off:off + w],
                        lhsT=kt_z[r[0]:r[1], h, jsl],
                        rhs=qt_z[r[0]:r[1], h, i0:i0 + w],
                        start=True, stop=True)
                sc_tiles[jt] = sc

            emit_scores(0)
            for jt in range(8):
                for f in fills.get(jt, []):
                    f()
                if jt < 7:
                    emit_scores(jt + 1)
                sc = sc_tiles[jt]
                if sc is None:
                    continue
                Dg = i0 - 128 * jt
                wA, wB = Ws[jt]
                p = pexp.tile([128, 1024], bf16, tag="p")
                if wA and wB:
                    nc.scalar.activation(p, sc, EXP)
                else:
                    off = 0 if wA else 512
                    nc.scalar.activation(p[:, off:off + 512],
                                         sc[:, off:off + 512], EXP)
                # post-exp ALiBi decay: p *= E_h on the columns with
                # nonzero bias (16-bit DVE multiply, off the PSUM loop).
                # Full width: beyond the live band the E table is zero,
                # which also zeroes the stale/garbage p columns that the
                # trimmed score matmul and exp never wrote -- so the PV
                # below can stay full-width with a single start flag.
                for half, h in ((0, hA), (1, hB)):
                    w = Ws[jt][half]
                    c0 = max(0, -Dg)
                    if w == 0 or Dg <= -512 or c0 >= 512:
                        continue
                    off = half * 512
                    nc.vector.tensor_tensor(
                        out=p[:, off + c0:off + 512],
                        in0=p[:, off + c0:off + 512],
                        in1=etab[:, h, Dg + c0:Dg + 512],
                        op=MULT)
                for half, h in ((0, hA), (1, hB)):
                    if Ws[jt][half] == 0:
                        continue
                    # fused PV + row-sums (M=65: 64 ctx rows + sums row)
                    off = half * 512
                    nc.tensor.matmul(
                        pvs[0:65, off:off + 512],
                        lhsT=v_sb[:, jt, h, :],
                        rhs=p[:, off:off + 512],
                        start=(jt == first_live[half]), stop=(jt == 7))

            if fast_norm:
                # normalize straight out of PSUM (no evac wait).  Emission
                # order matters: all DVE copies+recips first, then the
                # gpsimd broadcasts, then the TTs -- otherwise a TT waiting
                # on gpsimd stalls the DVE FIFO behind it.
                recip_h, rb_h = [None, None], [None, None]
                for half in (0, 1):
                    cs = slice(half * 512, (half + 1) * 512)
                    sums_h = small.tile([1, 512], f32, tag=f"sums{half}")
                    # ACT is idle right after the last exp; DVE still
                    # drains STT work -- copy sums there
                    nc.scalar.copy(sums_h, pvs[64:65, cs])
                    recip_h[half] = small.tile([1, 512], f32,
                                               tag=f"recip{half}",
                                               name=f"recip{half}")
                    nc.vector.reciprocal_approx_fast(recip_h[half], sums_h)
                for half in (0, 1):
                    rb_h[half] = small.tile([64, 512], f32, tag=f"rb{half}", bufs=1,
                                            name=f"rb{half}")
                    nc.gpsimd.partition_broadcast(rb_h[half], recip_h[half],
                                                  channels=64)
                for half, off in ((0, 0), (1, 64)):
                    cs = slice(half * 512, (half + 1) * 512)
                    nc.vector.tensor_tensor(
                        out=ctx_sb[off:off + 64, pair, i0:i0 + 512],
                        in0=pvs[0:64, cs], in1=rb_h[half], op=MULT)
            else:
                # Evacuate PSUM in one copy so the normalization chain runs
                # off the pvs-reuse critical path.
                pvs_sb = small.tile([65, 1024], f32, tag="pvs_sb", bufs=1)
                if ic == 1:   # ic1 groups are DVE-heavy; evacuate via ACT
                    nc.scalar.copy(pvs_sb, pvs[0:65, :])
                else:
                    nc.vector.tensor_copy(pvs_sb, pvs[0:65, :])
                sums_sb = small.tile([1, 1024], f32, tag="sums")
                nc.vector.tensor_copy(sums_sb, pvs_sb[64:65, :])
                recip = small.tile([1, 1024], f32, tag="recip")
                nc.vector.reciprocal_approx_fast(recip, sums_sb)
                rb = small.tile([64, 1024], f32, tag="rb", bufs=1)
                nc.gpsimd.partition_broadcast(rb, recip, channels=64)
                for half, off in ((0, 0), (1, 64)):
                    nc.vector.tensor_tensor(
                        out=ctx_sb[off:off + 64, pair, i0:i0 + 512],
                        in0=rb[:, half * 512:(half + 1) * 512],
                        in1=pvs_sb[0:64, half * 512:(half + 1) * 512],
                        op=MULT)

        # ---- schedule --------------------------------------------------
        # (host orders heads so local pair 3 is the most banded)
        P = partial
        kt_chain(0, 0)
        qt_chain(0, 0, first=True)

        attn_group(0, 0, {
            0: [P(v_chunk, 0, 0)], 1: [P(v_chunk, 1, 0)],
            2: [P(v_chunk, 2, 0), P(gen_etab, 2)],
            3: [P(v_chunk, 3, 0), P(kt_chain, 0, 1)],
            4: [P(v_chunk, 4, 0)], 5: [P(v_chunk, 5, 0), P(gen_etab, 3)],
            6: [P(v_chunk, 6, 0), P(v_chunk, 7, 0)],
            7: [P(qt_chain, 0, 1)]})
        attn_group(0, 1, {
            0: [P(kt_chain, 1, 0)], 1: [P(v_chunk, 0, 1), P(gen_etab, 4)],
            2: [P(kt_chain, 1, 1)], 3: [P(v_chunk, 1, 1)],
            4: [P(qt_chain, 1, 0)], 5: [P(v_chunk, 2, 1)],
            6: [P(v_chunk, 3, 1)], 7: [P(qt_chain, 1, 1)]})
        attn_group(1, 0, {
            0: [P(v_chunk, 4, 1)], 1: [P(v_chunk, 5, 1), P(gen_etab, 5)],
            2: [P(v_chunk, 6, 1)], 3: [P(v_chunk, 7, 1)],
            4: [P(kt_chain, 2, 0)], 6: [P(kt_chain, 2, 1)]})
        attn_group(1, 1, {
            0: [P(qt_chain, 2, 0)], 2: [P(qt_chain, 2, 1)],
            4: [P(kt_chain, 3, 0)], 6: [P(kt_chain, 3, 1)]})
        attn_group(2, 0, {
            0: [P(qt_chain, 3, 0)], 2: [P(qt_chain, 3, 1)]})
        # output-projection partials start as soon as the needed ctx
        # norms have landed (pairs 0-2 ic0 after group (2,0)'s norm)
        attn_group(2, 1, {
            4: [P(op_partial, 0, 0)],
            5: [P(op_partial, 1, 0)],
            6: [P(op_partial, 2, 0)],
            7: [P(op_partial, 3, 0)]})
        attn_group(3, 0, {
            0: [P(op_partial, 4, 0)], 1: [P(op_partial, 5, 0)],
            2: [P(op_partial, 6, 0)], 3: [P(op_partial, 7, 0)],
            4: [P(op_partial, 0, 1)], 5: [P(op_partial, 1, 1)],
            6: [P(op_partial, 2, 1)], 7: [P(op_partial, 3, 1)]},
            fast_norm=True)
        attn_group(3, 1, {
            0: [P(op_partial, 4, 1)],
            1: [P(op_partial, 5, 1)],
            2: [P(op_partial, 6, 1), P(op_final, 0, 0)],
            3: [P(op_partial, 7, 1), P(op_final, 1, 0)],
            4: [P(op_final, 2, 0)], 5: [P(op_final, 3, 0)],
            6: [P(op_final, 4, 0), P(op_final, 5, 0)],
            7: [P(op_final, 6, 0), P(op_final, 7, 0)]},
            fast_norm=True)
        for mt in range(8):
            op_final(mt, 1)

    nc.compile()
    return nc


def _get_nc():
    if "nc" not in _CACHE:
        _CACHE["nc"] = _build_nc()
    return _CACHE["nc"]


def _pack_x(xT, half):
    # [D, S] -> [128, 8, 512] with [p, kt, m] = xT[kt*128+p, half*512+m]
    a = xT.reshape(8, 128, S)[:, :, half * 512:(half + 1) * 512]
    return np.ascontiguousarray(a.transpose(1, 0, 2))


def _pack_xv(xT, half):
    # [D, S] -> [128, 4, 8, 128] with [p, q4, kt, m] =
    #   xT[kt*128+p, half*512 + q4*128 + m]
    a = xT.reshape(8, 128, S)[:, :, half * 512:(half + 1) * 512]
    a = a.reshape(8, 128, 4, 128)
    return np.ascontiguousarray(a.transpose(1, 2, 0, 3))


def _make_in_maps(q, k, v, Wq, Wout):
    q = np.asarray(q, dtype=np.float32)
    k = np.asarray(k, dtype=np.float32)
    v = np.asarray(v, dtype=np.float32)
    Wq = np.asarray(Wq, dtype=np.float32)
    Wout = np.asarray(Wout, dtype=np.float32)

    slopes = _alibi_slopes(H)
    icent = (np.arange(S, dtype=np.float32) - 512.0)

    in_maps = []
    for c in range(NCORES):
        b, par = c // 2, c % 2
        # heads sharded even/odd so the banded-attention savings (small
        # heads have big ALiBi slopes) spread across all cores; ordered
        # so the most-banded pair is processed LAST (shortest tail)
        hsel = [g + par for g in (4, 6, 8, 10, 12, 14, 0, 2)]
        dsel = np.concatenate([np.arange(h * HD, (h + 1) * HD) for h in hsel])
        wq_l = Wq[dsel, :].T.astype(np.float16)        # [D, DL]
        wo_l = Wout[:, dsel].T.astype(np.float16)      # [DL, D]
        qT = q[b].T.astype(np.float16)
        kT = k[b].T.astype(np.float16)
        vT = v[b].T.astype(np.float16)
        # wq packed [p, mt, kt, m] = wq_l[kt*128+p, mt*128+m]
        wq_p = wq_l.reshape(8, 128, 4, 128).transpose(1, 2, 0, 3)
        # wo packed [p, ct, o] = wo_l[ct*128+p, o]
        wo_p = wo_l.reshape(4, 128, D).transpose(1, 0, 2)
        # s~ = fp16-rounded slopes; -s~ feeds the on-device E-table
        # generation as the activation scale (per-core: parity selects
        # the slopes)
        st = slopes[hsel].astype(np.float16).astype(np.float32)  # [HL]
        in_maps.append({
            "xq0": _pack_x(qT, 0), "xq1": _pack_x(qT, 1),
            "xk0": _pack_x(kT, 0), "xk1": _pack_x(kT, 1),
            "xv0": _pack_xv(vT, 0), "xv1": _pack_xv(vT, 1),
            "wq": np.ascontiguousarray(wq_p),
            "wo": np.ascontiguousarray(wo_p),
            "negs": np.ascontiguousarray(-st[None, :]),
        })
    return in_maps


def kernel(q, k, v, mask, Wq, Wout):
    from concourse.bass_utils import run_bass_kernel_spmd

    nc = _get_nc()
    in_maps = _make_in_maps(q, k, v, Wq, Wout)
    res = run_bass_kernel_spmd(nc, in_maps, core_ids=list(range(NCORES)))

    out = np.empty((B, S, D), dtype=np.float32)
    for b in range(B):
        out[b] = (res.results[2 * b]["out"].T.astype(np.float32)
                  + res.results[2 * b + 1]["out"].T.astype(np.float32))
    return out
